# revision 6
# baseline (speedup 1.0000x reference)
"""HSTU dense-transformer Trainium2 kernel, 8-core SPMD.

Sharding: tokens row-sharded (512/core) for pointwise+matmul ops; attention
head-sharded (2 heads x 2 batches per core).  x^T replicated via AllGather,
attention output redistributed via AllToAll.  All matmuls fp32r (tf32-like).

kernel(**inputs) takes the full unsharded inputs (as in reference.setup_inputs)
and returns (log_feats [B,S,H], pos_embs, neg_embs, loss_mask [B,S]).
"""
import os
import sys
import time
from contextlib import ExitStack

sys.path.insert(0, "/opt/trn_rl_repo")

import numpy as np

import concourse.bass as bass
import concourse.tile as tile
from concourse import bacc, mybir
from concourse.masks import make_identity

F32 = mybir.dt.float32
F32R = mybir.dt.float32r
AF = mybir.ActivationFunctionType
OP = mybir.AluOpType

NC = 8            # cores
B, S, H, NHL = 2, 2048, 1024, 4   # NHL = layers
T = B * S         # 4096 tokens
R = T // NC       # 512 rows/core
KT = H // 128     # 8 k-tiles
HD = 64           # head dim
L = 4
N_ITEM, N_USER = 100000, 10000
SCALE = HD ** -0.5
DEBUG = bool(int(os.environ.get("BASSK_DEBUG", "0")))

_CACHE = {}


def _build_module():
    nc = bacc.Bacc("TRN2", target_bir_lowering=False, debug=False, num_devices=NC)
    RG = [list(range(NC))]

    def din(name, shape, dt=F32):
        return nc.dram_tensor(name, shape, dt, kind="ExternalInput")

    def dout(name, shape, dt=F32):
        return nc.dram_tensor(name, shape, dt, kind="ExternalOutput")

    # ---- external inputs (per core) ----
    eT = {s: din(f"{s}_eT", [H, R]) for s in ("item", "user", "pos", "neg")}
    w_item = din("w_item", [H, H])
    w_user = din("w_user", [H, H])
    b_item32 = din("b_item32", [KT, 128])
    b_user32 = din("b_user32", [KT, 128])
    b_item_pl = din("b_item_pl", [KT, 128])
    wqkv = din("wqkv", [L, H, 384])
    bqkv = din("bqkv", [L, 3, 128])
    wu = din("wu", [L, H, H])
    bu = din("bu", [L, KT, 128])
    wt = din("wt", [L, H, H])
    bt = din("bt", [L, KT, 128])
    lng = din("lng", [L, KT, 128])
    lnb = din("lnb", [L, KT, 128])
    lastg = din("lastg", [KT, 128])
    lastb = din("lastb", [KT, 128])
    cosx_d = din("cosx", [128, S])
    sinx_d = din("sinx", [128, S])
    kmaskx_d = din("kmaskx", [128, T])
    onesp_d = din("onesp", [128, 1])
    onesb_d = din("onesb", [1, 128])

    # ---- outputs ----
    log_T = dout("log_T", [H, R])
    pos_T = dout("pos_T", [H, R])
    neg_T = dout("neg_T", [H, R])
    dbg = {}
    if DEBUG:
        dbg["x0"] = dout("dbg_x0", [NC, H, R])
        dbg["qr0"] = dout("dbg_qr0", [128, T])
        dbg["kr0"] = dout("dbg_kr0", [128, T])
        dbg["a2a0"] = dout("dbg_a2a0", [NC, 128, R])
        dbg["g0"] = dout("dbg_g0", [KT, 128, R])
        dbg["x1"] = dout("dbg_x1", [H, R])

    # ---- internal DRAM ----
    ag_in = [nc.dram_tensor(f"ag_in{l}", [H, R], F32) for l in range(L)]
    x_all = [nc.dram_tensor(f"x_all{l}", [NC, H, R], F32, addr_space="Shared")
             for l in range(L)]
    a2a_i = [nc.dram_tensor(f"a2a_i{l}", [NC, 128, R], F32) for l in range(L)]
    a2a_o = [nc.dram_tensor(f"a2a_o{l}", [NC, 128, R], F32) for l in range(L)]

    with tile.TileContext(nc) as tc, ExitStack() as ctx:
        const = ctx.enter_context(tc.tile_pool(name="const", bufs=1))
        big = ctx.enter_context(tc.tile_pool(name="big", bufs=1))
        wq_pool = ctx.enter_context(tc.tile_pool(name="wqp", bufs=1))
        xs_pool = ctx.enter_context(tc.tile_pool(name="xs", bufs=3))
        ws_pool = ctx.enter_context(tc.tile_pool(name="ws", bufs=2))
        ev_pool = ctx.enter_context(tc.tile_pool(name="ev", bufs=3))
        rt_pool = ctx.enter_context(tc.tile_pool(name="rt", bufs=2))
        wt_pool = ctx.enter_context(tc.tile_pool(name="wt", bufs=3))
        oc_pool = ctx.enter_context(tc.tile_pool(name="oc", bufs=2))
        vt_pool = ctx.enter_context(tc.tile_pool(name="vt", bufs=2))
        sm_pool = ctx.enter_context(tc.tile_pool(name="sm", bufs=1))
        xn_pool = ctx.enter_context(tc.tile_pool(name="xn", bufs=2))
        ps_mm = ctx.enter_context(tc.tile_pool(name="psmm", bufs=3, space="PSUM"))
        ps_s = ctx.enter_context(tc.tile_pool(name="pss", bufs=2, space="PSUM"))
        ps_o = ctx.enter_context(tc.tile_pool(name="pso", bufs=2, space="PSUM"))

        # ---------------- constants ----------------
        ident = const.tile([128, 128], F32, name="ident")
        make_identity(nc, ident[:])
        eps_t = const.tile([1, 1], F32, name="eps_t")
        nc.any.memset(eps_t[:], 1e-8)
        onesp = const.tile([128, 1], F32R, name="onesp")
        nc.sync.dma_start(onesp[:], onesp_d.ap().bitcast(F32R))
        onesb = const.tile([1, 128], F32R, name="onesb")
        nc.sync.dma_start(onesb[:], onesb_d.ap().bitcast(F32R))
        cosx = const.tile([128, S], F32, name="cosx")
        nc.sync.dma_start(cosx[:], cosx_d[:])
        sinx = const.tile([128, S], F32, name="sinx")
        nc.sync.dma_start(sinx[:], sinx_d[:])
        bqkv_sb = const.tile([128, L, 3], F32, name="bqkv_sb")
        nc.sync.dma_start(bqkv_sb[:], bqkv.ap().rearrange("l c p -> p l c"))
        bu_sb = const.tile([128, L, KT], F32, name="bu_sb")
        nc.sync.dma_start(bu_sb[:], bu.ap().rearrange("l m p -> p l m"))
        bt_sb = const.tile([128, L, KT], F32, name="bt_sb")
        nc.sync.dma_start(bt_sb[:], bt.ap().rearrange("l m p -> p l m"))
        lng_sb = const.tile([128, L, KT], F32, name="lng_sb")
        nc.sync.dma_start(lng_sb[:], lng.ap().rearrange("l m p -> p l m"))
        lnb_sb = const.tile([128, L, KT], F32, name="lnb_sb")
        nc.sync.dma_start(lnb_sb[:], lnb.ap().rearrange("l m p -> p l m"))
        lastg_sb = const.tile([128, KT], F32, name="lastg_sb")
        nc.sync.dma_start(lastg_sb[:], lastg.ap().rearrange("m p -> p m"))
        lastb_sb = const.tile([128, KT], F32, name="lastb_sb")
        nc.sync.dma_start(lastb_sb[:], lastb.ap().rearrange("m p -> p m"))
        bi32_sb = const.tile([128, KT], F32, name="bi32_sb")
        nc.sync.dma_start(bi32_sb[:], b_item32.ap().rearrange("m p -> p m"))
        bu32_sb = const.tile([128, KT], F32, name="bu32_sb")
        nc.sync.dma_start(bu32_sb[:], b_user32.ap().rearrange("m p -> p m"))
        bipl_sb = const.tile([128, KT], F32, name="bipl_sb")
        nc.sync.dma_start(bipl_sb[:], b_item_pl.ap().rearrange("m p -> p m"))

        def load_eT(src, tag):
            t = big.tile([128, KT, R], F32R, name=f"eT_{src}", tag=tag)
            nc.sync.dma_start(
                t[:], eT[src].ap().rearrange("(kt p) r -> p kt r", p=128).bitcast(F32R))
            return t

        def wstream(dram_2d, m, lview=None):
            """Load [128, KT, 128] k-major slice of a [H, H] weight (cols 128m..)."""
            t = ws_pool.tile([128, KT, 128], F32R, name="wsm", tag="wsm")
            ap = dram_2d if lview is None else dram_2d
            nc.sync.dma_start(
                t[:],
                ap.rearrange("(kt p) m -> p kt m", p=128)[:, :, 128 * m:128 * (m + 1)]
                .bitcast(F32R))
            return t

        # ---------------- phase 0 ----------------
        eT_item = load_eT("item", "tagA")
        eT_pos = load_eT("pos", "tagB")
        eT_neg = load_eT("neg", "tagC")
        eT_user = load_eT("user", "tagD")
        xacc = big.tile([128, KT, R], F32, name="xacc", tag="tagF")

        for m in range(KT):
            wm = wstream(w_item.ap(), m)
            for src, et in (("item", eT_item), ("pos", eT_pos), ("neg", eT_neg)):
                ps = ps_mm.tile([128, 512], F32, name="ps0", tag="mm")
                for k in range(KT):
                    nc.tensor.matmul(ps[:], wm[:, k], et[:, k],
                                     start=(k == 0), stop=(k == KT - 1))
                if src == "item":
                    nc.scalar.activation(xacc[:, m], ps[:], AF.Relu,
                                         bias=bi32_sb[:, m:m + 1], scale=32.0)
                else:
                    ot = ev_pool.tile([128, 512], F32, name="evt", tag="ev")
                    nc.scalar.activation(ot[:], ps[:], AF.Relu,
                                         bias=bipl_sb[:, m:m + 1])
                    dst = pos_T if src == "pos" else neg_T
                    nc.sync.dma_start(dst.ap()[128 * m:128 * (m + 1), :], ot[:])
        for m in range(KT):
            wm = wstream(w_user.ap(), m)
            ps = ps_mm.tile([128, 512], F32, name="ps0u", tag="mm")
            for k in range(KT):
                nc.tensor.matmul(ps[:], wm[:, k], eT_user[:, k],
                                 start=(k == 0), stop=(k == KT - 1))
            ut = ev_pool.tile([128, 512], F32, name="evtu", tag="ev")
            nc.scalar.activation(ut[:], ps[:], AF.Relu,
                                 bias=bu32_sb[:, m:m + 1], scale=32.0)
            nc.vector.tensor_add(xacc[:, m], xacc[:, m], ut[:])
            nc.sync.dma_start(ag_in[0].ap()[128 * m:128 * (m + 1), :], xacc[:, m])
        nc.gpsimd.collective_compute(
            "AllGather", OP.bypass, replica_groups=RG,
            ins=[ag_in[0].ap()], outs=[x_all[0].ap()])
        if DEBUG:
            nc.sync.dma_start(dbg["x0"].ap(), x_all[0].ap())

        # ---------------- layers ----------------
        for l in range(L):
            # --- U^T (needs only own block ag_in[l]; overlaps prior AG) ---
            xblk = big.tile([128, KT, R], F32R, name=f"xblk{l}", tag="xblk")
            nc.sync.dma_start(
                xblk[:],
                ag_in[l].ap().rearrange("(kt p) r -> p kt r", p=128).bitcast(F32R))
            UsT = big.tile([128, KT, R], F32, name=f"UsT{l}", tag="tagD")
            for m in range(KT):
                wm = wstream(wu.ap()[l], m)
                ps = ps_mm.tile([128, 512], F32, name="psu", tag="mm")
                for k in range(KT):
                    nc.tensor.matmul(ps[:], wm[:, k], xblk[:, k],
                                     start=(k == 0), stop=(k == KT - 1))
                nc.scalar.activation(UsT[:, m], ps[:], AF.Silu,
                                     bias=bu_sb[:, l, m:m + 1])

            # --- QKV over all token blocks ---
            wq_sb = wq_pool.tile([128, KT, 384], F32R, name=f"wq{l}", tag="wq")
            nc.sync.dma_start(
                wq_sb[:],
                wqkv.ap()[l].rearrange("(kt p) m -> p kt m", p=128).bitcast(F32R))
            QrT = big.tile([128, T], F32R, name=f"QrT{l}", tag="tagA")
            KrT = big.tile([128, T], F32R, name=f"KrT{l}", tag="tagB")
            vnorm = big.tile([128, 32, 128], F32R, name=f"vn{l}", tag="tagC")

            for r in range(NC):
                ps3 = [ps_mm.tile([128, 512], F32, name=f"psq{m}", tag="mm")
                       for m in range(3)]
                for k in range(KT):
                    t = xs_pool.tile([128, 512], F32R, name="xk", tag="xk")
                    nc.sync.dma_start(
                        t[:], x_all[l].ap()[r][128 * k:128 * (k + 1), :].bitcast(F32R))
                    for m in range(3):  # 0=V 1=Q 2=K
                        nc.tensor.matmul(ps3[m][:], wq_sb[:, k, 128 * m:128 * (m + 1)],
                                         t[:], start=(k == 0), stop=(k == KT - 1))
                ts = slice(512 * r, 512 * (r + 1))
                for m in range(3):
                    ps = ps3[m]
                    tmp = ev_pool.tile([128, 512], F32, name="qevt", tag="ev")
                    nc.scalar.activation(tmp[:], ps[:], AF.Identity,
                                         bias=bqkv_sb[:, l, m:m + 1])
                    if m == 0:
                        km = ev_pool.tile([128, 512], F32, name="km", tag="ev")
                        nc.sync.dma_start(km[:], kmaskx_d.ap()[:, ts])
                        vm = vt_pool.tile([128, 512], F32, name="vm", tag="vm")
                        nc.vector.tensor_mul(vm[:], tmp[:], km[:])
                        for q in range(4):
                            pst = ps_s.tile([128, 128], F32, name="pst", tag="s")
                            nc.tensor.transpose(pst[:], vm[:, 128 * q:128 * (q + 1)],
                                                ident[:])
                            nc.vector.tensor_copy(vnorm[:, 4 * r + q], pst[:])
                    else:
                        dstT = QrT if m == 1 else KrT
                        pl = 512 * (r % 4)
                        rq = rt_pool.tile([128, 512], F32, name="rq", tag="rq")
                        for bp in (0, 64):
                            nc.scalar.mul(rq[bp:bp + 32, :], tmp[bp + 32:bp + 64, :],
                                          -1.0)
                            nc.vector.tensor_copy(rq[bp + 32:bp + 64, :],
                                                  tmp[bp:bp + 32, :])
                        t1 = rt_pool.tile([128, 512], F32, name="rt1", tag="rt1")
                        nc.vector.tensor_mul(t1[:], tmp[:], cosx[:, pl:pl + 512])
                        nc.vector.tensor_mul(rq[:], rq[:], sinx[:, pl:pl + 512])
                        nc.vector.tensor_add(dstT[:, ts], t1[:], rq[:])
            if DEBUG and l == 0:
                nc.sync.dma_start(dbg["qr0"].ap(), QrT[:].bitcast(F32))
                nc.sync.dma_start(dbg["kr0"].ap(), KrT[:].bitcast(F32))

            # --- attention ---
            for b in range(B):
                for j in range(4):
                    och = oc_pool.tile([128, 512], F32, name="och", tag="och")
                    for hh in range(2):
                        hs = slice(64 * hh, 64 * hh + 64)
                        pso = ps_o.tile([64, 512], F32, name="pso", tag="o")
                        nt = 4 * (j + 1)
                        for t in range(nt):
                            pss = ps_s.tile([128, 512], F32, name="pss", tag="s")
                            nc.tensor.matmul(
                                pss[:],
                                KrT[hs, S * b + 128 * t: S * b + 128 * (t + 1)],
                                QrT[hs, S * b + 512 * j: S * b + 512 * (j + 1)],
                                start=True, stop=True)
                            wsb = wt_pool.tile([128, 512], F32R, name="wsb", tag="w")
                            nc.scalar.activation(wsb[:], pss[:], AF.Sigmoid,
                                                 scale=SCALE)
                            d = t - 4 * j
                            if d >= 0:
                                # keep where qq - kk - 128d >= 0 else 0
                                nc.gpsimd.affine_select(
                                    wsb[:], wsb[:], pattern=[[1, 512]],
                                    compare_op=OP.is_ge, fill=0.0,
                                    base=-128 * d, channel_multiplier=-1)
                            nc.tensor.matmul(pso[:], vnorm[:, 16 * b + t, hs],
                                             wsb[:], start=(t == 0),
                                             stop=(t == nt - 1))
                        nc.vector.tensor_copy(och[hs, :], pso[:])
                    nc.sync.dma_start(a2a_i[l].ap()[4 * b + j], och[:])
            nc.gpsimd.collective_compute(
                "AllToAll", OP.bypass, replica_groups=RG,
                ins=[a2a_i[l].ap()], outs=[a2a_o[l].ap()])
            if DEBUG and l == 0:
                nc.sync.dma_start(dbg["a2a0"].ap(), a2a_o[l].ap())

            # --- LN stats on o (channels across partitions x m) ---
            og = big.tile([128, KT, R], F32R, name=f"og{l}", tag="tagF")
            nc.sync.dma_start(
                og[:],
                a2a_o[l].ap().rearrange("c p r -> p c r").bitcast(F32R))
            ps_sum = ps_mm.tile([1, 512], F32, name="ps_sum", tag="mm")
            ps_sq = ps_mm.tile([1, 512], F32, name="ps_sq", tag="mm")
            for m in range(KT):
                sq = sm_pool.tile([128, 512], F32R, name="sq", tag="sq", bufs=2)
                nc.vector.tensor_mul(sq[:], og[:, m], og[:, m])
                nc.tensor.matmul(ps_sum[:], onesp[:], og[:, m],
                                 start=(m == 0), stop=(m == KT - 1))
                nc.tensor.matmul(ps_sq[:], onesp[:], sq[:],
                                 start=(m == 0), stop=(m == KT - 1))
            mean = sm_pool.tile([1, 512], F32, name="mean", tag="ln1")
            nc.scalar.mul(mean[:], ps_sum[:], 1.0 / H)
            ex2 = sm_pool.tile([1, 512], F32, name="ex2", tag="ln2")
            nc.scalar.mul(ex2[:], ps_sq[:], 1.0 / H)
            var = sm_pool.tile([1, 512], F32, name="var", tag="ln3")
            nc.vector.tensor_mul(var[:], mean[:], mean[:])
            nc.vector.tensor_sub(var[:], ex2[:], var[:])
            std = sm_pool.tile([1, 512], F32, name="std", tag="ln2")
            nc.scalar.activation(std[:], var[:], AF.Sqrt, bias=eps_t[:])
            rstd = sm_pool.tile([1, 512], F32, name="rstd", tag="ln3")
            nc.vector.reciprocal(rstd[:], std[:])
            stats2 = sm_pool.tile([1, 1024], F32R, name="stats2", tag="ln6")
            nc.vector.tensor_copy(stats2[:, 0:512], rstd[:])
            nc.vector.tensor_mul(stats2[:, 512:1024], mean[:], rstd[:])
            bps1 = ps_mm.tile([128, 512], F32, name="bps1", tag="mm")
            nc.tensor.matmul(bps1[:], onesb[:], stats2[:, 0:512], start=True,
                             stop=True)
            bps2 = ps_mm.tile([128, 512], F32, name="bps2", tag="mm")
            nc.tensor.matmul(bps2[:], onesb[:], stats2[:, 512:1024], start=True,
                             stop=True)
            bc_rstd = sm_pool.tile([128, 512], F32, name="bc_rstd", tag="bc1")
            nc.vector.tensor_copy(bc_rstd[:], bps1[:])
            bc_mr = sm_pool.tile([128, 512], F32, name="bc_mr", tag="bc2")
            nc.vector.tensor_copy(bc_mr[:], bps2[:])

            # --- normalize, gate (in place into og -> becomes gT) ---
            for m in range(KT):
                t1 = ev_pool.tile([128, 512], F32, name="lnt", tag="ev")
                nc.vector.tensor_mul(t1[:], og[:, m], bc_rstd[:])
                nc.vector.tensor_sub(t1[:], t1[:], bc_mr[:])
                nc.vector.tensor_scalar(t1[:], t1[:], lng_sb[:, l, m:m + 1],
                                        lnb_sb[:, l, m:m + 1], OP.mult, OP.add)
                nc.vector.tensor_mul(og[:, m], t1[:], UsT[:, m])
            if DEBUG and l == 0:
                nc.sync.dma_start(
                    dbg["g0"].ap().rearrange("m p r -> p m r"), og[:].bitcast(F32))

            # --- out^T + residual ---
            last = (l == L - 1)
            if last:
                xfin = big.tile([128, KT, R], F32R, name="xfin", tag="tagC")
            for m in range(KT):
                wm = wstream(wt.ap()[l], m)
                ps = ps_mm.tile([128, 512], F32, name="pst2", tag="mm")
                for k in range(KT):
                    nc.tensor.matmul(ps[:], wm[:, k], og[:, k],
                                     start=(k == 0), stop=(k == KT - 1))
                xn = xn_pool.tile([128, 512], F32, name="xn", tag="xn")
                nc.scalar.activation(xn[:], ps[:], AF.Identity,
                                     bias=bt_sb[:, l, m:m + 1])
                if last:
                    nc.vector.tensor_add(xfin[:, m], xn[:],
                                         xblk[:, m].bitcast(F32))
                else:
                    nc.vector.tensor_add(xn[:], xn[:], xblk[:, m].bitcast(F32))
                    nc.sync.dma_start(
                        ag_in[l + 1].ap()[128 * m:128 * (m + 1), :], xn[:])
            if not last:
                nc.gpsimd.collective_compute(
                    "AllGather", OP.bypass, replica_groups=RG,
                    ins=[ag_in[l + 1].ap()], outs=[x_all[l + 1].ap()])
                if DEBUG and l == 0:
                    nc.sync.dma_start(dbg["x1"].ap(), ag_in[1].ap())

        # ---------------- final layernorm ----------------
        ps_sum = ps_mm.tile([1, 512], F32, name="fsum", tag="mm")
        ps_sq = ps_mm.tile([1, 512], F32, name="fsq", tag="mm")
        for m in range(KT):
            sq = sm_pool.tile([128, 512], F32R, name="fsqt", tag="sq", bufs=2)
            nc.vector.tensor_mul(sq[:], xfin[:, m], xfin[:, m])
            nc.tensor.matmul(ps_sum[:], onesp[:], xfin[:, m],
                             start=(m == 0), stop=(m == KT - 1))
            nc.tensor.matmul(ps_sq[:], onesp[:], sq[:],
                             start=(m == 0), stop=(m == KT - 1))
        mean = sm_pool.tile([1, 512], F32, name="fmean", tag="ln1")
        nc.scalar.mul(mean[:], ps_sum[:], 1.0 / H)
        ex2 = sm_pool.tile([1, 512], F32, name="fex2", tag="ln2")
        nc.scalar.mul(ex2[:], ps_sq[:], 1.0 / H)
        var = sm_pool.tile([1, 512], F32, name="fvar", tag="ln3")
        nc.vector.tensor_mul(var[:], mean[:], mean[:])
        nc.vector.tensor_sub(var[:], ex2[:], var[:])
        std = sm_pool.tile([1, 512], F32, name="fstd", tag="ln2")
        nc.scalar.activation(std[:], var[:], AF.Sqrt, bias=eps_t[:])
        rstd = sm_pool.tile([1, 512], F32, name="frstd", tag="ln3")
        nc.vector.reciprocal(rstd[:], std[:])
        stats2 = sm_pool.tile([1, 1024], F32R, name="fstats2", tag="ln6")
        nc.vector.tensor_copy(stats2[:, 0:512], rstd[:])
        nc.vector.tensor_mul(stats2[:, 512:1024], mean[:], rstd[:])
        bps1 = ps_mm.tile([128, 512], F32, name="fbps1", tag="mm")
        nc.tensor.matmul(bps1[:], onesb[:], stats2[:, 0:512], start=True, stop=True)
        bps2 = ps_mm.tile([128, 512], F32, name="fbps2", tag="mm")
        nc.tensor.matmul(bps2[:], onesb[:], stats2[:, 512:1024], start=True,
                         stop=True)
        bc_rstd = sm_pool.tile([128, 512], F32, name="fbc_rstd", tag="bc1")
        nc.vector.tensor_copy(bc_rstd[:], bps1[:])
        bc_mr = sm_pool.tile([128, 512], F32, name="fbc_mr", tag="bc2")
        nc.vector.tensor_copy(bc_mr[:], bps2[:])
        for m in range(KT):
            t1 = ev_pool.tile([128, 512], F32, name="flnt", tag="ev")
            nc.vector.tensor_mul(t1[:], xfin[:, m], bc_rstd[:])
            nc.vector.tensor_sub(t1[:], t1[:], bc_mr[:])
            nc.vector.tensor_scalar(t1[:], t1[:], lastg_sb[:, m:m + 1],
                                    lastb_sb[:, m:m + 1], OP.mult, OP.add)
            nc.sync.dma_start(log_T.ap()[128 * m:128 * (m + 1), :], t1[:])

    nc.compile()
    return nc


def _host_prep(inputs):
    """Build per-core in_maps from full inputs."""
    inp = {k: np.asarray(v) for k, v in inputs.items()}
    item_emb = np.ascontiguousarray(inp["item_emb"], np.float32)
    user_emb = np.ascontiguousarray(inp["user_emb"], np.float32)
    mask = inp["mask"]
    ui = inp["user_item"]
    idx_item = np.clip(np.where(mask == 1, ui, 0), 0, item_emb.shape[0] - 1).reshape(T)
    idx_user = np.clip(np.where(mask == 2, ui, 0), 0, user_emb.shape[0] - 1).reshape(T)
    idx_pos = np.clip(inp["pos_seqs"], 0, item_emb.shape[0] - 1).reshape(T)
    idx_neg = np.clip(inp["neg_seqs"], 0, item_emb.shape[0] - 1).reshape(T)

    f32 = lambda x: np.ascontiguousarray(x, np.float32)
    proj_w = f32(inp["proj_w"])     # [L, H, 4H]
    proj_b = f32(inp["proj_b"])     # [L, 4H]
    wqkv = np.empty((L, H, 384), np.float32)
    bqkv = np.empty((L, 3, 128), np.float32)

    pos_of_t = np.arange(S)
    inv_freq = 1.0 / (10000.0 ** (np.arange(0, HD, 2, dtype=np.float64) / HD))
    freqs = pos_of_t[None, :] * inv_freq[:, None]          # [32, S]
    cos32 = np.cos(freqs).astype(np.float32)
    sin32 = np.sin(freqs).astype(np.float32)
    cosx = np.tile(cos32, (4, 1))                           # [128, S]
    sinx = np.tile(sin32, (4, 1))
    kmaskx = np.broadcast_to(
        (mask.reshape(T) != 0).astype(np.float32)[None, :], (128, T)).copy()
    shared = {
        "w_item": f32(inp["itemdnn_w"]),
        "w_user": f32(inp["userdnn_w"]),
        "b_item32": f32(32.0 * inp["itemdnn_b"]).reshape(KT, 128),
        "b_user32": f32(32.0 * inp["userdnn_b"]).reshape(KT, 128),
        "b_item_pl": f32(inp["itemdnn_b"]).reshape(KT, 128),
        "wu": np.ascontiguousarray(proj_w[:, :, :H]),
        "bu": proj_b[:, :H].reshape(L, KT, 128).copy(),
        "wt": f32(inp["tr_w"]),
        "bt": f32(inp["tr_b"]).reshape(L, KT, 128),
        "lng": f32(inp["ln_g"]).reshape(L, KT, 128),
        "lnb": f32(inp["ln_b"]).reshape(L, KT, 128),
        "lastg": f32(inp["last_g"]).reshape(KT, 128),
        "lastb": f32(inp["last_b"]).reshape(KT, 128),
        "cosx": cosx, "sinx": sinx, "kmaskx": kmaskx,
        "onesp": np.ones((128, 1), np.float32),
        "onesb": np.ones((1, 128), np.float32),
    }

    in_maps = []
    for c in range(NC):
        for part in range(3):   # V, Q, K blocks for this core's heads
            base = H * (part + 1) + 128 * c
            wqkv[:, :, 128 * part:128 * (part + 1)] = proj_w[:, :, base:base + 128]
            bqkv[:, part, :] = proj_b[:, base:base + 128]
        sl = slice(c * R, (c + 1) * R)
        m = dict(shared)
        m["wqkv"] = wqkv.copy()
        m["bqkv"] = bqkv.copy()
        m["item_eT"] = np.ascontiguousarray(item_emb[idx_item[sl]].T)
        m["user_eT"] = np.ascontiguousarray(user_emb[idx_user[sl]].T)
        m["pos_eT"] = np.ascontiguousarray(item_emb[idx_pos[sl]].T)
        m["neg_eT"] = np.ascontiguousarray(item_emb[idx_neg[sl]].T)
        in_maps.append(m)
    return in_maps


# ---------------- runner (jit built once, supports repeat timing) ----------------

def _get_runner():
    if "runner" in _CACHE:
        return _CACHE["runner"]
    import jax
    from jax.sharding import Mesh, PartitionSpec
    from concourse import bass2jax
    from concourse.bass2jax import (_bass_exec_p, install_neuronx_cc_hook,
                                    partition_id_tensor)
    import concourse.mybir as mybir_

    nc = _build_module()
    install_neuronx_cc_hook()

    partition_name = (nc.partition_id_tensor.name
                      if nc.partition_id_tensor else None)
    in_names, out_names, out_avals = [], [], []
    for alloc in nc.m.functions[0].allocations:
        if not isinstance(alloc, mybir_.MemoryLocationSet):
            continue
        name = alloc.memorylocations[0].name
        if alloc.kind == "ExternalInput":
            if name != partition_name:
                in_names.append(name)
        elif alloc.kind == "ExternalOutput":
            out_names.append(name)
            out_avals.append(jax.core.ShapedArray(
                tuple(alloc.tensor_shape), mybir_.dt.np(alloc.dtype)))
    n_params = len(in_names)
    all_names = list(in_names) + out_names
    if partition_name is not None:
        all_names.append(partition_name)

    def _body(*args):
        operands = list(args)
        if partition_name is not None:
            operands.append(partition_id_tensor())
        outs = _bass_exec_p.bind(
            *operands,
            out_avals=tuple(out_avals),
            in_names=tuple(all_names),
            out_names=tuple(out_names),
            lowering_input_output_aliases=(),
            sim_require_finite=True,
            sim_require_nnan=True,
            nc=nc,
        )
        return tuple(outs)

    devices = jax.devices()[:NC]
    mesh = Mesh(np.asarray(devices), ("core",))
    n_outs = len(out_names)
    sharded = jax.jit(
        jax.shard_map(_body, mesh=mesh,
                      in_specs=(PartitionSpec("core"),) * (n_params + n_outs),
                      out_specs=(PartitionSpec("core"),) * n_outs,
                      check_vma=False),
        donate_argnums=tuple(range(n_params, n_params + n_outs)),
        keep_unused=True)

    runner = (sharded, in_names, out_names, out_avals, n_params, mesh)
    _CACHE["runner"] = runner
    return runner


def _run(in_maps, n_timing_iters=0):
    import jax
    from jax.sharding import NamedSharding, PartitionSpec
    sharded, in_names, out_names, out_avals, n_params, mesh = _get_runner()
    concat_in = [
        np.concatenate([np.ascontiguousarray(in_maps[c][nm]) for c in range(NC)], 0)
        for nm in in_names]
    zeros = [np.zeros((NC * a.shape[0], *a.shape[1:]), a.dtype) for a in out_avals]
    out = sharded(*concat_in, *zeros)
    jax.block_until_ready(out)
    best_ns = None
    if n_timing_iters:
        sh = NamedSharding(mesh, PartitionSpec("core"))
        dev_in = [jax.device_put(x, sh) for x in concat_in]
        jax.block_until_ready(dev_in)
        times = []
        for _ in range(n_timing_iters):
            z = [jax.device_put(np.zeros((NC * a.shape[0], *a.shape[1:]), a.dtype),
                                sh)
                 for a in out_avals]
            jax.block_until_ready(z)
            t0 = time.perf_counter()
            o2 = sharded(*dev_in, *z)
            jax.block_until_ready(o2)
            times.append(time.perf_counter() - t0)
        print("timing samples ms:", [f"{t*1e3:.2f}" for t in times])
        best_ns = int(min(times) * 1e9)
    results = [
        {nm: np.asarray(out[i]).reshape(NC, *out_avals[i].shape)[c]
         for i, nm in enumerate(out_names)}
        for c in range(NC)]
    return results, best_ns


def _assemble(results, inputs):
    log = np.concatenate([r["log_T"].T for r in results], 0).reshape(B, S, H)
    pos = np.concatenate([r["pos_T"].T for r in results], 0).reshape(B, S, H)
    neg = np.concatenate([r["neg_T"].T for r in results], 0).reshape(B, S, H)
    loss_mask = np.asarray(inputs["next_mask"]) == 1
    return log, pos, neg, loss_mask


def kernel(**inputs):
    in_maps = _host_prep(inputs)
    results, _ = _run(in_maps)
    return _assemble(results, inputs)


def kernel_timed(n_iters=5, **inputs):
    in_maps = _host_prep(inputs)
    results, best_ns = _run(in_maps, n_timing_iters=n_iters)
    return _assemble(results, inputs), best_ns, results


# revision 7
# speedup vs baseline: 45.3600x; 45.3600x over previous
"""HSTU dense-transformer Trainium2 kernel, 8-core SPMD.

Sharding: tokens row-sharded (512/core) for pointwise+matmul ops; attention
head-sharded (2 heads x 2 batches per core).  x^T replicated via AllGather,
attention output redistributed via AllToAll.  All matmuls fp32r (tf32-like).

kernel(**inputs) takes the full unsharded inputs (as in reference.setup_inputs)
and returns (log_feats [B,S,H], pos_embs, neg_embs, loss_mask [B,S]).
"""
import os
import sys
import time
from contextlib import ExitStack

sys.path.insert(0, "/opt/trn_rl_repo")

import numpy as np

import concourse.bass as bass
import concourse.tile as tile
from concourse import bacc, mybir
from concourse.masks import make_identity

F32 = mybir.dt.float32
F32R = mybir.dt.float32r
AF = mybir.ActivationFunctionType
OP = mybir.AluOpType

NC = 8            # cores
B, S, H, NHL = 2, 2048, 1024, 4   # NHL = layers
T = B * S         # 4096 tokens
R = T // NC       # 512 rows/core
KT = H // 128     # 8 k-tiles
HD = 64           # head dim
L = 4
N_ITEM, N_USER = 100000, 10000
SCALE = HD ** -0.5
DEBUG = bool(int(os.environ.get("BASSK_DEBUG", "0")))

_CACHE = {}


def _build_module():
    nc = bacc.Bacc("TRN2", target_bir_lowering=False, debug=False, num_devices=NC)
    RG = [list(range(NC))]

    def din(name, shape, dt=F32):
        return nc.dram_tensor(name, shape, dt, kind="ExternalInput")

    def dout(name, shape, dt=F32):
        return nc.dram_tensor(name, shape, dt, kind="ExternalOutput")

    # ---- external inputs (per core) ----
    eT = {s: din(f"{s}_eT", [H, R]) for s in ("item", "user", "pos", "neg")}
    w_item = din("w_item", [H, H])
    w_user = din("w_user", [H, H])
    b_item32 = din("b_item32", [KT, 128])
    b_user32 = din("b_user32", [KT, 128])
    b_item_pl = din("b_item_pl", [KT, 128])
    wqkv = din("wqkv", [L, H, 384])
    bqkv = din("bqkv", [L, 3, 128])
    wu = din("wu", [L, H, H])
    bu = din("bu", [L, KT, 128])
    wt = din("wt", [L, H, H])
    bt = din("bt", [L, KT, 128])
    lng = din("lng", [L, KT, 128])
    lnb = din("lnb", [L, KT, 128])
    lastg = din("lastg", [KT, 128])
    lastb = din("lastb", [KT, 128])
    cosx_d = din("cosx", [128, S])
    sinx_d = din("sinx", [128, S])
    kmaskx_d = din("kmaskx", [128, T])
    onesp_d = din("onesp", [128, 1])
    onesb_d = din("onesb", [1, 128])

    # ---- outputs ----
    log_T = dout("log_T", [H, R])
    pos_T = dout("pos_T", [H, R])
    neg_T = dout("neg_T", [H, R])
    dbg = {}
    if DEBUG:
        dbg["x0"] = dout("dbg_x0", [NC, H, R])
        dbg["qr0"] = dout("dbg_qr0", [128, T])
        dbg["kr0"] = dout("dbg_kr0", [128, T])
        dbg["a2a0"] = dout("dbg_a2a0", [NC, 128, R])
        dbg["g0"] = dout("dbg_g0", [KT, 128, R])
        dbg["x1"] = dout("dbg_x1", [H, R])

    # ---- internal DRAM ----
    ag_in = [nc.dram_tensor(f"ag_in{l}", [H, R], F32) for l in range(L)]
    x_all = [nc.dram_tensor(f"x_all{l}", [NC, H, R], F32, addr_space="Shared")
             for l in range(L)]
    a2a_i = [nc.dram_tensor(f"a2a_i{l}", [NC, 128, R], F32) for l in range(L)]
    a2a_o = [nc.dram_tensor(f"a2a_o{l}", [NC, 128, R], F32) for l in range(L)]

    with tile.TileContext(nc) as tc, ExitStack() as ctx:
        const = ctx.enter_context(tc.tile_pool(name="const", bufs=1))
        big = ctx.enter_context(tc.tile_pool(name="big", bufs=1))
        wq_pool = ctx.enter_context(tc.tile_pool(name="wqp", bufs=1))
        xs_pool = ctx.enter_context(tc.tile_pool(name="xs", bufs=3))
        ws_pool = ctx.enter_context(tc.tile_pool(name="ws", bufs=2))
        ev_pool = ctx.enter_context(tc.tile_pool(name="ev", bufs=3))
        rt_pool = ctx.enter_context(tc.tile_pool(name="rt", bufs=2))
        wt_pool = ctx.enter_context(tc.tile_pool(name="wt", bufs=3))
        oc_pool = ctx.enter_context(tc.tile_pool(name="oc", bufs=2))
        vt_pool = ctx.enter_context(tc.tile_pool(name="vt", bufs=2))
        sm_pool = ctx.enter_context(tc.tile_pool(name="sm", bufs=1))
        xn_pool = ctx.enter_context(tc.tile_pool(name="xn", bufs=2))
        ps_mm = ctx.enter_context(tc.tile_pool(name="psmm", bufs=3, space="PSUM"))
        ps_s = ctx.enter_context(tc.tile_pool(name="pss", bufs=2, space="PSUM"))
        ps_o = ctx.enter_context(tc.tile_pool(name="pso", bufs=2, space="PSUM"))

        # ---------------- constants ----------------
        ident = const.tile([128, 128], F32, name="ident")
        make_identity(nc, ident[:])
        eps_t = const.tile([1, 1], F32, name="eps_t")
        nc.any.memset(eps_t[:], 1e-8)
        onesp = const.tile([128, 1], F32R, name="onesp")
        nc.sync.dma_start(onesp[:], onesp_d.ap().bitcast(F32R))
        onesb = const.tile([1, 128], F32R, name="onesb")
        nc.sync.dma_start(onesb[:], onesb_d.ap().bitcast(F32R))
        cosx = const.tile([128, S], F32, name="cosx")
        nc.sync.dma_start(cosx[:], cosx_d[:])
        sinx = const.tile([128, S], F32, name="sinx")
        nc.sync.dma_start(sinx[:], sinx_d[:])
        bqkv_sb = const.tile([128, L, 3], F32, name="bqkv_sb")
        nc.sync.dma_start(bqkv_sb[:], bqkv.ap().rearrange("l c p -> p l c"))
        bu_sb = const.tile([128, L, KT], F32, name="bu_sb")
        nc.sync.dma_start(bu_sb[:], bu.ap().rearrange("l m p -> p l m"))
        bt_sb = const.tile([128, L, KT], F32, name="bt_sb")
        nc.sync.dma_start(bt_sb[:], bt.ap().rearrange("l m p -> p l m"))
        lng_sb = const.tile([128, L, KT], F32, name="lng_sb")
        nc.sync.dma_start(lng_sb[:], lng.ap().rearrange("l m p -> p l m"))
        lnb_sb = const.tile([128, L, KT], F32, name="lnb_sb")
        nc.sync.dma_start(lnb_sb[:], lnb.ap().rearrange("l m p -> p l m"))
        lastg_sb = const.tile([128, KT], F32, name="lastg_sb")
        nc.sync.dma_start(lastg_sb[:], lastg.ap().rearrange("m p -> p m"))
        lastb_sb = const.tile([128, KT], F32, name="lastb_sb")
        nc.sync.dma_start(lastb_sb[:], lastb.ap().rearrange("m p -> p m"))
        bi32_sb = const.tile([128, KT], F32, name="bi32_sb")
        nc.sync.dma_start(bi32_sb[:], b_item32.ap().rearrange("m p -> p m"))
        bu32_sb = const.tile([128, KT], F32, name="bu32_sb")
        nc.sync.dma_start(bu32_sb[:], b_user32.ap().rearrange("m p -> p m"))
        bipl_sb = const.tile([128, KT], F32, name="bipl_sb")
        nc.sync.dma_start(bipl_sb[:], b_item_pl.ap().rearrange("m p -> p m"))

        def load_eT(src, tag):
            t = big.tile([128, KT, R], F32R, name=f"eT_{src}", tag=tag)
            nc.sync.dma_start(
                t[:], eT[src].ap().rearrange("(kt p) r -> p kt r", p=128).bitcast(F32R))
            return t

        def wstream(dram_2d, m, lview=None):
            """Load [128, KT, 128] k-major slice of a [H, H] weight (cols 128m..)."""
            t = ws_pool.tile([128, KT, 128], F32R, name="wsm", tag="wsm")
            ap = dram_2d if lview is None else dram_2d
            nc.sync.dma_start(
                t[:],
                ap.rearrange("(kt p) m -> p kt m", p=128)[:, :, 128 * m:128 * (m + 1)]
                .bitcast(F32R))
            return t

        # ---------------- phase 0 ----------------
        eT_item = load_eT("item", "tagA")
        eT_pos = load_eT("pos", "tagB")
        eT_neg = load_eT("neg", "tagC")
        eT_user = load_eT("user", "tagD")
        xacc = big.tile([128, KT, R], F32, name="xacc", tag="tagF")

        for m in range(KT):
            wm = wstream(w_item.ap(), m)
            for src, et in (("item", eT_item), ("pos", eT_pos), ("neg", eT_neg)):
                ps = ps_mm.tile([128, 512], F32, name="ps0", tag="mm")
                for k in range(KT):
                    nc.tensor.matmul(ps[:], wm[:, k], et[:, k],
                                     start=(k == 0), stop=(k == KT - 1))
                if src == "item":
                    nc.scalar.activation(xacc[:, m], ps[:], AF.Relu,
                                         bias=bi32_sb[:, m:m + 1], scale=32.0)
                else:
                    ot = ev_pool.tile([128, 512], F32, name="evt", tag="ev")
                    nc.scalar.activation(ot[:], ps[:], AF.Relu,
                                         bias=bipl_sb[:, m:m + 1])
                    dst = pos_T if src == "pos" else neg_T
                    nc.sync.dma_start(dst.ap()[128 * m:128 * (m + 1), :], ot[:])
        for m in range(KT):
            wm = wstream(w_user.ap(), m)
            ps = ps_mm.tile([128, 512], F32, name="ps0u", tag="mm")
            for k in range(KT):
                nc.tensor.matmul(ps[:], wm[:, k], eT_user[:, k],
                                 start=(k == 0), stop=(k == KT - 1))
            ut = ev_pool.tile([128, 512], F32, name="evtu", tag="ev")
            nc.scalar.activation(ut[:], ps[:], AF.Relu,
                                 bias=bu32_sb[:, m:m + 1], scale=32.0)
            nc.vector.tensor_add(xacc[:, m], xacc[:, m], ut[:])
            nc.sync.dma_start(ag_in[0].ap()[128 * m:128 * (m + 1), :], xacc[:, m])
        nc.gpsimd.collective_compute(
            "AllGather", OP.bypass, replica_groups=RG,
            ins=[ag_in[0].ap()], outs=[x_all[0].ap()])
        if DEBUG:
            nc.sync.dma_start(dbg["x0"].ap(), x_all[0].ap())

        # ---------------- layers ----------------
        for l in range(L):
            # --- U^T (needs only own block ag_in[l]; overlaps prior AG) ---
            xblk = big.tile([128, KT, R], F32R, name=f"xblk{l}", tag="xblk")
            nc.sync.dma_start(
                xblk[:],
                ag_in[l].ap().rearrange("(kt p) r -> p kt r", p=128).bitcast(F32R))
            UsT = big.tile([128, KT, R], F32, name=f"UsT{l}", tag="tagD")
            for m in range(KT):
                wm = wstream(wu.ap()[l], m)
                ps = ps_mm.tile([128, 512], F32, name="psu", tag="mm")
                for k in range(KT):
                    nc.tensor.matmul(ps[:], wm[:, k], xblk[:, k],
                                     start=(k == 0), stop=(k == KT - 1))
                nc.scalar.activation(UsT[:, m], ps[:], AF.Silu,
                                     bias=bu_sb[:, l, m:m + 1])

            # --- QKV over all token blocks ---
            wq_sb = wq_pool.tile([128, KT, 384], F32R, name=f"wq{l}", tag="wq")
            nc.sync.dma_start(
                wq_sb[:],
                wqkv.ap()[l].rearrange("(kt p) m -> p kt m", p=128).bitcast(F32R))
            QrT = big.tile([128, T], F32R, name=f"QrT{l}", tag="tagA")
            KrT = big.tile([128, T], F32R, name=f"KrT{l}", tag="tagB")
            vnorm = big.tile([128, 32, 128], F32R, name=f"vn{l}", tag="tagC")

            for r in range(NC):
                ps3 = [ps_mm.tile([128, 512], F32, name=f"psq{m}", tag="mm")
                       for m in range(3)]
                for k in range(KT):
                    t = xs_pool.tile([128, 512], F32R, name="xk", tag="xk")
                    nc.sync.dma_start(
                        t[:], x_all[l].ap()[r][128 * k:128 * (k + 1), :].bitcast(F32R))
                    for m in range(3):  # 0=V 1=Q 2=K
                        nc.tensor.matmul(ps3[m][:], wq_sb[:, k, 128 * m:128 * (m + 1)],
                                         t[:], start=(k == 0), stop=(k == KT - 1))
                ts = slice(512 * r, 512 * (r + 1))
                for m in range(3):
                    ps = ps3[m]
                    tmp = ev_pool.tile([128, 512], F32, name="qevt", tag="ev")
                    nc.scalar.activation(tmp[:], ps[:], AF.Identity,
                                         bias=bqkv_sb[:, l, m:m + 1])
                    if m == 0:
                        km = ev_pool.tile([128, 512], F32, name="km", tag="ev")
                        nc.sync.dma_start(km[:], kmaskx_d.ap()[:, ts])
                        vm = vt_pool.tile([128, 512], F32, name="vm", tag="vm")
                        nc.vector.tensor_mul(vm[:], tmp[:], km[:])
                        for q in range(4):
                            pst = ps_s.tile([128, 128], F32, name="pst", tag="s")
                            nc.tensor.transpose(pst[:], vm[:, 128 * q:128 * (q + 1)],
                                                ident[:])
                            nc.vector.tensor_copy(vnorm[:, 4 * r + q], pst[:])
                    else:
                        dstT = QrT if m == 1 else KrT
                        pl = 512 * (r % 4)
                        rq = rt_pool.tile([128, 512], F32, name="rq", tag="rq")
                        for bp in (0, 64):
                            nc.scalar.mul(rq[bp:bp + 32, :], tmp[bp + 32:bp + 64, :],
                                          -1.0)
                            nc.vector.tensor_copy(rq[bp + 32:bp + 64, :],
                                                  tmp[bp:bp + 32, :])
                        t1 = rt_pool.tile([128, 512], F32, name="rt1", tag="rt1")
                        nc.vector.tensor_mul(t1[:], tmp[:], cosx[:, pl:pl + 512])
                        nc.vector.tensor_mul(rq[:], rq[:], sinx[:, pl:pl + 512])
                        nc.vector.tensor_add(dstT[:, ts], t1[:], rq[:])
            if DEBUG and l == 0:
                nc.sync.dma_start(dbg["qr0"].ap(), QrT[:].bitcast(F32))
                nc.sync.dma_start(dbg["kr0"].ap(), KrT[:].bitcast(F32))

            # --- attention ---
            for b in range(B):
                for j in range(4):
                    och = oc_pool.tile([128, 512], F32, name="och", tag="och")
                    for hh in range(2):
                        hs = slice(64 * hh, 64 * hh + 64)
                        pso = ps_o.tile([64, 512], F32, name="pso", tag="o")
                        nt = 4 * (j + 1)
                        for t in range(nt):
                            pss = ps_s.tile([128, 512], F32, name="pss", tag="s")
                            nc.tensor.matmul(
                                pss[:],
                                KrT[hs, S * b + 128 * t: S * b + 128 * (t + 1)],
                                QrT[hs, S * b + 512 * j: S * b + 512 * (j + 1)],
                                start=True, stop=True)
                            wsb = wt_pool.tile([128, 512], F32R, name="wsb", tag="w")
                            nc.scalar.activation(wsb[:], pss[:], AF.Sigmoid,
                                                 scale=SCALE)
                            d = t - 4 * j
                            if d >= 0:
                                # keep where qq - kk - 128d >= 0 else 0
                                nc.gpsimd.affine_select(
                                    wsb[:], wsb[:], pattern=[[1, 512]],
                                    compare_op=OP.is_ge, fill=0.0,
                                    base=-128 * d, channel_multiplier=-1)
                            nc.tensor.matmul(pso[:], vnorm[:, 16 * b + t, hs],
                                             wsb[:], start=(t == 0),
                                             stop=(t == nt - 1))
                        nc.vector.tensor_copy(och[hs, :], pso[:])
                    nc.sync.dma_start(a2a_i[l].ap()[4 * b + j], och[:])
            nc.gpsimd.collective_compute(
                "AllToAll", OP.bypass, replica_groups=RG,
                ins=[a2a_i[l].ap()], outs=[a2a_o[l].ap()])
            if DEBUG and l == 0:
                nc.sync.dma_start(dbg["a2a0"].ap(), a2a_o[l].ap())

            # --- LN stats on o (channels across partitions x m) ---
            og = big.tile([128, KT, R], F32R, name=f"og{l}", tag="tagF")
            nc.sync.dma_start(
                og[:],
                a2a_o[l].ap().rearrange("c p r -> p c r").bitcast(F32R))
            ps_sum = ps_mm.tile([1, 512], F32, name="ps_sum", tag="mm")
            ps_sq = ps_mm.tile([1, 512], F32, name="ps_sq", tag="mm")
            for m in range(KT):
                sq = sm_pool.tile([128, 512], F32R, name="sq", tag="sq", bufs=2)
                nc.vector.tensor_mul(sq[:], og[:, m], og[:, m])
                nc.tensor.matmul(ps_sum[:], onesp[:], og[:, m],
                                 start=(m == 0), stop=(m == KT - 1))
                nc.tensor.matmul(ps_sq[:], onesp[:], sq[:],
                                 start=(m == 0), stop=(m == KT - 1))
            mean = sm_pool.tile([1, 512], F32, name="mean", tag="ln1")
            nc.scalar.mul(mean[:], ps_sum[:], 1.0 / H)
            ex2 = sm_pool.tile([1, 512], F32, name="ex2", tag="ln2")
            nc.scalar.mul(ex2[:], ps_sq[:], 1.0 / H)
            var = sm_pool.tile([1, 512], F32, name="var", tag="ln3")
            nc.vector.tensor_mul(var[:], mean[:], mean[:])
            nc.vector.tensor_sub(var[:], ex2[:], var[:])
            std = sm_pool.tile([1, 512], F32, name="std", tag="ln2")
            nc.scalar.activation(std[:], var[:], AF.Sqrt, bias=eps_t[:])
            rstd = sm_pool.tile([1, 512], F32, name="rstd", tag="ln3")
            nc.vector.reciprocal(rstd[:], std[:])
            stats2 = sm_pool.tile([1, 1024], F32R, name="stats2", tag="ln6")
            nc.vector.tensor_copy(stats2[:, 0:512], rstd[:])
            nc.vector.tensor_mul(stats2[:, 512:1024], mean[:], rstd[:])
            bps1 = ps_mm.tile([128, 512], F32, name="bps1", tag="mm")
            nc.tensor.matmul(bps1[:], onesb[:], stats2[:, 0:512], start=True,
                             stop=True)
            bps2 = ps_mm.tile([128, 512], F32, name="bps2", tag="mm")
            nc.tensor.matmul(bps2[:], onesb[:], stats2[:, 512:1024], start=True,
                             stop=True)
            bc_rstd = sm_pool.tile([128, 512], F32, name="bc_rstd", tag="bc1")
            nc.vector.tensor_copy(bc_rstd[:], bps1[:])
            bc_mr = sm_pool.tile([128, 512], F32, name="bc_mr", tag="bc2")
            nc.vector.tensor_copy(bc_mr[:], bps2[:])

            # --- normalize, gate (in place into og -> becomes gT) ---
            for m in range(KT):
                t1 = ev_pool.tile([128, 512], F32, name="lnt", tag="ev")
                nc.vector.tensor_mul(t1[:], og[:, m], bc_rstd[:])
                nc.vector.tensor_sub(t1[:], t1[:], bc_mr[:])
                nc.vector.tensor_scalar(t1[:], t1[:], lng_sb[:, l, m:m + 1],
                                        lnb_sb[:, l, m:m + 1], OP.mult, OP.add)
                nc.vector.tensor_mul(og[:, m], t1[:], UsT[:, m])
            if DEBUG and l == 0:
                nc.sync.dma_start(
                    dbg["g0"].ap().rearrange("m p r -> p m r"), og[:].bitcast(F32))

            # --- out^T + residual ---
            last = (l == L - 1)
            if last:
                xfin = big.tile([128, KT, R], F32R, name="xfin", tag="tagC")
            for m in range(KT):
                wm = wstream(wt.ap()[l], m)
                ps = ps_mm.tile([128, 512], F32, name="pst2", tag="mm")
                for k in range(KT):
                    nc.tensor.matmul(ps[:], wm[:, k], og[:, k],
                                     start=(k == 0), stop=(k == KT - 1))
                xn = xn_pool.tile([128, 512], F32, name="xn", tag="xn")
                nc.scalar.activation(xn[:], ps[:], AF.Identity,
                                     bias=bt_sb[:, l, m:m + 1])
                if last:
                    nc.vector.tensor_add(xfin[:, m], xn[:],
                                         xblk[:, m].bitcast(F32))
                else:
                    nc.vector.tensor_add(xn[:], xn[:], xblk[:, m].bitcast(F32))
                    nc.sync.dma_start(
                        ag_in[l + 1].ap()[128 * m:128 * (m + 1), :], xn[:])
            if not last:
                nc.gpsimd.collective_compute(
                    "AllGather", OP.bypass, replica_groups=RG,
                    ins=[ag_in[l + 1].ap()], outs=[x_all[l + 1].ap()])
                if DEBUG and l == 0:
                    nc.sync.dma_start(dbg["x1"].ap(), ag_in[1].ap())

        # ---------------- final layernorm ----------------
        ps_sum = ps_mm.tile([1, 512], F32, name="fsum", tag="mm")
        ps_sq = ps_mm.tile([1, 512], F32, name="fsq", tag="mm")
        for m in range(KT):
            sq = sm_pool.tile([128, 512], F32R, name="fsqt", tag="sq", bufs=2)
            nc.vector.tensor_mul(sq[:], xfin[:, m], xfin[:, m])
            nc.tensor.matmul(ps_sum[:], onesp[:], xfin[:, m],
                             start=(m == 0), stop=(m == KT - 1))
            nc.tensor.matmul(ps_sq[:], onesp[:], sq[:],
                             start=(m == 0), stop=(m == KT - 1))
        mean = sm_pool.tile([1, 512], F32, name="fmean", tag="ln1")
        nc.scalar.mul(mean[:], ps_sum[:], 1.0 / H)
        ex2 = sm_pool.tile([1, 512], F32, name="fex2", tag="ln2")
        nc.scalar.mul(ex2[:], ps_sq[:], 1.0 / H)
        var = sm_pool.tile([1, 512], F32, name="fvar", tag="ln3")
        nc.vector.tensor_mul(var[:], mean[:], mean[:])
        nc.vector.tensor_sub(var[:], ex2[:], var[:])
        std = sm_pool.tile([1, 512], F32, name="fstd", tag="ln2")
        nc.scalar.activation(std[:], var[:], AF.Sqrt, bias=eps_t[:])
        rstd = sm_pool.tile([1, 512], F32, name="frstd", tag="ln3")
        nc.vector.reciprocal(rstd[:], std[:])
        stats2 = sm_pool.tile([1, 1024], F32R, name="fstats2", tag="ln6")
        nc.vector.tensor_copy(stats2[:, 0:512], rstd[:])
        nc.vector.tensor_mul(stats2[:, 512:1024], mean[:], rstd[:])
        bps1 = ps_mm.tile([128, 512], F32, name="fbps1", tag="mm")
        nc.tensor.matmul(bps1[:], onesb[:], stats2[:, 0:512], start=True, stop=True)
        bps2 = ps_mm.tile([128, 512], F32, name="fbps2", tag="mm")
        nc.tensor.matmul(bps2[:], onesb[:], stats2[:, 512:1024], start=True,
                         stop=True)
        bc_rstd = sm_pool.tile([128, 512], F32, name="fbc_rstd", tag="bc1")
        nc.vector.tensor_copy(bc_rstd[:], bps1[:])
        bc_mr = sm_pool.tile([128, 512], F32, name="fbc_mr", tag="bc2")
        nc.vector.tensor_copy(bc_mr[:], bps2[:])
        for m in range(KT):
            t1 = ev_pool.tile([128, 512], F32, name="flnt", tag="ev")
            nc.vector.tensor_mul(t1[:], xfin[:, m], bc_rstd[:])
            nc.vector.tensor_sub(t1[:], t1[:], bc_mr[:])
            nc.vector.tensor_scalar(t1[:], t1[:], lastg_sb[:, m:m + 1],
                                    lastb_sb[:, m:m + 1], OP.mult, OP.add)
            nc.sync.dma_start(log_T.ap()[128 * m:128 * (m + 1), :], t1[:])

    nc.compile()
    return nc


def _host_prep(inputs):
    """Build per-core in_maps from full inputs."""
    inp = {k: np.asarray(v) for k, v in inputs.items()}
    item_emb = np.ascontiguousarray(inp["item_emb"], np.float32)
    user_emb = np.ascontiguousarray(inp["user_emb"], np.float32)
    mask = inp["mask"]
    ui = inp["user_item"]
    idx_item = np.clip(np.where(mask == 1, ui, 0), 0, item_emb.shape[0] - 1).reshape(T)
    idx_user = np.clip(np.where(mask == 2, ui, 0), 0, user_emb.shape[0] - 1).reshape(T)
    idx_pos = np.clip(inp["pos_seqs"], 0, item_emb.shape[0] - 1).reshape(T)
    idx_neg = np.clip(inp["neg_seqs"], 0, item_emb.shape[0] - 1).reshape(T)

    f32 = lambda x: np.ascontiguousarray(x, np.float32)
    proj_w = f32(inp["proj_w"])     # [L, H, 4H]
    proj_b = f32(inp["proj_b"])     # [L, 4H]
    wqkv = np.empty((L, H, 384), np.float32)
    bqkv = np.empty((L, 3, 128), np.float32)

    pos_of_t = np.arange(S)
    inv_freq = 1.0 / (10000.0 ** (np.arange(0, HD, 2, dtype=np.float64) / HD))
    freqs = pos_of_t[None, :] * inv_freq[:, None]          # [32, S]
    cos32 = np.cos(freqs).astype(np.float32)
    sin32 = np.sin(freqs).astype(np.float32)
    cosx = np.tile(cos32, (4, 1))                           # [128, S]
    sinx = np.tile(sin32, (4, 1))
    kmaskx = np.broadcast_to(
        (mask.reshape(T) != 0).astype(np.float32)[None, :], (128, T)).copy()
    shared = {
        "w_item": f32(inp["itemdnn_w"]),
        "w_user": f32(inp["userdnn_w"]),
        "b_item32": f32(32.0 * inp["itemdnn_b"]).reshape(KT, 128),
        "b_user32": f32(32.0 * inp["userdnn_b"]).reshape(KT, 128),
        "b_item_pl": f32(inp["itemdnn_b"]).reshape(KT, 128),
        "wu": np.ascontiguousarray(proj_w[:, :, :H]),
        "bu": proj_b[:, :H].reshape(L, KT, 128).copy(),
        "wt": f32(inp["tr_w"]),
        "bt": f32(inp["tr_b"]).reshape(L, KT, 128),
        "lng": f32(inp["ln_g"]).reshape(L, KT, 128),
        "lnb": f32(inp["ln_b"]).reshape(L, KT, 128),
        "lastg": f32(inp["last_g"]).reshape(KT, 128),
        "lastb": f32(inp["last_b"]).reshape(KT, 128),
        "cosx": cosx, "sinx": sinx, "kmaskx": kmaskx,
        "onesp": np.ones((128, 1), np.float32),
        "onesb": np.ones((1, 128), np.float32),
    }

    in_maps = []
    for c in range(NC):
        for part in range(3):   # V, Q, K blocks for this core's heads
            base = H * (part + 1) + 128 * c
            wqkv[:, :, 128 * part:128 * (part + 1)] = proj_w[:, :, base:base + 128]
            bqkv[:, part, :] = proj_b[:, base:base + 128]
        sl = slice(c * R, (c + 1) * R)
        m = dict(shared)
        m["wqkv"] = wqkv.copy()
        m["bqkv"] = bqkv.copy()
        m["item_eT"] = np.ascontiguousarray(item_emb[idx_item[sl]].T)
        m["user_eT"] = np.ascontiguousarray(user_emb[idx_user[sl]].T)
        m["pos_eT"] = np.ascontiguousarray(item_emb[idx_pos[sl]].T)
        m["neg_eT"] = np.ascontiguousarray(item_emb[idx_neg[sl]].T)
        in_maps.append(m)
    return in_maps


# ---------------- runner (jit built once, supports repeat timing) ----------------

def _get_runner():
    if "runner" in _CACHE:
        return _CACHE["runner"]
    import jax
    from jax.sharding import Mesh, PartitionSpec
    from concourse import bass2jax
    from concourse.bass2jax import (_bass_exec_p, install_neuronx_cc_hook,
                                    partition_id_tensor)
    import concourse.mybir as mybir_

    nc = _build_module()
    _CACHE["nc"] = nc
    install_neuronx_cc_hook()

    partition_name = (nc.partition_id_tensor.name
                      if nc.partition_id_tensor else None)
    in_names, out_names, out_avals = [], [], []
    for alloc in nc.m.functions[0].allocations:
        if not isinstance(alloc, mybir_.MemoryLocationSet):
            continue
        name = alloc.memorylocations[0].name
        if alloc.kind == "ExternalInput":
            if name != partition_name:
                in_names.append(name)
        elif alloc.kind == "ExternalOutput":
            out_names.append(name)
            out_avals.append(jax.core.ShapedArray(
                tuple(alloc.tensor_shape), mybir_.dt.np(alloc.dtype)))
    n_params = len(in_names)
    all_names = list(in_names) + out_names
    if partition_name is not None:
        all_names.append(partition_name)

    def _body(*args):
        operands = list(args)
        if partition_name is not None:
            operands.append(partition_id_tensor())
        outs = _bass_exec_p.bind(
            *operands,
            out_avals=tuple(out_avals),
            in_names=tuple(all_names),
            out_names=tuple(out_names),
            lowering_input_output_aliases=(),
            sim_require_finite=True,
            sim_require_nnan=True,
            nc=nc,
        )
        return tuple(outs)

    devices = jax.devices()[:NC]
    mesh = Mesh(np.asarray(devices), ("core",))
    n_outs = len(out_names)
    sharded = jax.jit(
        jax.shard_map(_body, mesh=mesh,
                      in_specs=(PartitionSpec("core"),) * (n_params + n_outs),
                      out_specs=(PartitionSpec("core"),) * n_outs,
                      check_vma=False),
        donate_argnums=tuple(range(n_params, n_params + n_outs)),
        keep_unused=True)

    runner = (sharded, in_names, out_names, out_avals, n_params, mesh)
    _CACHE["runner"] = runner
    return runner


def _run(in_maps, n_timing_iters=0):
    import jax
    from jax.sharding import NamedSharding, PartitionSpec
    sharded, in_names, out_names, out_avals, n_params, mesh = _get_runner()
    concat_in = [
        np.concatenate([np.ascontiguousarray(in_maps[c][nm]) for c in range(NC)], 0)
        for nm in in_names]
    zeros = [np.zeros((NC * a.shape[0], *a.shape[1:]), a.dtype) for a in out_avals]
    out = sharded(*concat_in, *zeros)
    jax.block_until_ready(out)
    best_ns = None
    if n_timing_iters:
        sh = NamedSharding(mesh, PartitionSpec("core"))
        dev_in = [jax.device_put(x, sh) for x in concat_in]
        jax.block_until_ready(dev_in)
        times = []
        for _ in range(n_timing_iters):
            z = [jax.device_put(np.zeros((NC * a.shape[0], *a.shape[1:]), a.dtype),
                                sh)
                 for a in out_avals]
            jax.block_until_ready(z)
            t0 = time.perf_counter()
            o2 = sharded(*dev_in, *z)
            jax.block_until_ready(o2)
            times.append(time.perf_counter() - t0)
        print("timing samples ms:", [f"{t*1e3:.2f}" for t in times])
        best_ns = int(min(times) * 1e9)
    results = [
        {nm: np.asarray(out[i]).reshape(NC, *out_avals[i].shape)[c]
         for i, nm in enumerate(out_names)}
        for c in range(NC)]
    return results, best_ns


def _assemble(results, inputs):
    log = np.concatenate([r["log_T"].T for r in results], 0).reshape(B, S, H)
    pos = np.concatenate([r["pos_T"].T for r in results], 0).reshape(B, S, H)
    neg = np.concatenate([r["neg_T"].T for r in results], 0).reshape(B, S, H)
    loss_mask = np.asarray(inputs["next_mask"]) == 1
    return log, pos, neg, loss_mask


def kernel(**inputs):
    in_maps = _host_prep(inputs)
    results, _ = _run(in_maps)
    return _assemble(results, inputs)


def kernel_timed(n_iters=5, **inputs):
    in_maps = _host_prep(inputs)
    results, best_ns = _run(in_maps, n_timing_iters=n_iters)
    return _assemble(results, inputs), best_ns, results


# revision 8
# speedup vs baseline: 49.1073x; 1.0826x over previous
"""HSTU dense-transformer Trainium2 kernel, 8-core SPMD.

Sharding: tokens row-sharded (512/core) for pointwise+matmul ops; attention
head-sharded (2 heads x 2 batches per core).  x^T replicated via AllGather,
attention output redistributed via AllToAll.  All matmuls fp32r (tf32-like).

kernel(**inputs) takes the full unsharded inputs (as in reference.setup_inputs)
and returns (log_feats [B,S,H], pos_embs, neg_embs, loss_mask [B,S]).
"""
import os
import sys
import time
from contextlib import ExitStack

sys.path.insert(0, "/opt/trn_rl_repo")

import numpy as np

import concourse.bass as bass
import concourse.tile as tile
from concourse import bacc, mybir
from concourse.masks import make_identity

F32 = mybir.dt.float32
F32R = mybir.dt.float32r
BF16 = mybir.dt.bfloat16
AF = mybir.ActivationFunctionType
OP = mybir.AluOpType

NC = 8            # cores
B, S, H, NHL = 2, 2048, 1024, 4   # NHL = layers
T = B * S         # 4096 tokens
R = T // NC       # 512 rows/core
KT = H // 128     # 8 k-tiles
HD = 64           # head dim
L = 4
N_ITEM, N_USER = 100000, 10000
SCALE = HD ** -0.5
DEBUG = bool(int(os.environ.get("BASSK_DEBUG", "0")))

_CACHE = {}


def _build_module():
    nc = bacc.Bacc("TRN2", target_bir_lowering=False, debug=False, num_devices=NC)
    RG = [list(range(NC))]

    def din(name, shape, dt=F32):
        return nc.dram_tensor(name, shape, dt, kind="ExternalInput")

    def dout(name, shape, dt=F32):
        return nc.dram_tensor(name, shape, dt, kind="ExternalOutput")

    # ---- external inputs (per core) ----
    eT = {s: din(f"{s}_eT", [H, R]) for s in ("item", "user", "pos", "neg")}
    w_item = din("w_item", [H, H])
    w_user = din("w_user", [H, H])
    b_item32 = din("b_item32", [KT, 128])
    b_user32 = din("b_user32", [KT, 128])
    b_item_pl = din("b_item_pl", [KT, 128])
    wqkv = din("wqkv", [L, H, 384])
    bqkv = din("bqkv", [L, 3, 128])
    wu = din("wu", [L, H, H])
    bu = din("bu", [L, KT, 128])
    wt = din("wt", [L, H, H])
    bt = din("bt", [L, KT, 128])
    lng = din("lng", [L, KT, 128])
    lnb = din("lnb", [L, KT, 128])
    lastg = din("lastg", [KT, 128])
    lastb = din("lastb", [KT, 128])
    cosx_d = din("cosx", [128, S])
    sinx_d = din("sinx", [128, S])
    kmaskx_d = din("kmaskx", [128, T])
    onesp_d = din("onesp", [128, 1])
    onesb_d = din("onesb", [1, 128])

    # ---- outputs ----
    log_T = dout("log_T", [H, R])
    pos_T = dout("pos_T", [H, R])
    neg_T = dout("neg_T", [H, R])
    dbg = {}
    if DEBUG:
        dbg["x0"] = dout("dbg_x0", [NC, H, R])
        dbg["qr0"] = dout("dbg_qr0", [128, T])
        dbg["kr0"] = dout("dbg_kr0", [128, T])
        dbg["a2a0"] = dout("dbg_a2a0", [NC, 128, R])
        dbg["g0"] = dout("dbg_g0", [KT, 128, R])
        dbg["x1"] = dout("dbg_x1", [H, R])

    # ---- internal DRAM ----
    ag_in = [nc.dram_tensor(f"ag_in{l}", [H, R], F32) for l in range(L)]
    x_all = [nc.dram_tensor(f"x_all{l}", [NC, H, R], F32, addr_space="Shared")
             for l in range(L)]
    a2a_i = [nc.dram_tensor(f"a2a_i{l}", [NC, 128, R], F32) for l in range(L)]
    a2a_o = [nc.dram_tensor(f"a2a_o{l}", [NC, 128, R], F32) for l in range(L)]

    with tile.TileContext(nc) as tc, ExitStack() as ctx:
        const = ctx.enter_context(tc.tile_pool(name="const", bufs=1))
        big = ctx.enter_context(tc.tile_pool(name="big", bufs=1))
        wq_pool = ctx.enter_context(tc.tile_pool(name="wqp", bufs=1))
        xs_pool = ctx.enter_context(tc.tile_pool(name="xs", bufs=3))
        ws_pool = ctx.enter_context(tc.tile_pool(name="ws", bufs=2))
        ev_pool = ctx.enter_context(tc.tile_pool(name="ev", bufs=3))
        rt_pool = ctx.enter_context(tc.tile_pool(name="rt", bufs=2))
        wt_pool = ctx.enter_context(tc.tile_pool(name="wt", bufs=4))
        oc_pool = ctx.enter_context(tc.tile_pool(name="oc", bufs=2))
        vt_pool = ctx.enter_context(tc.tile_pool(name="vt", bufs=2))
        sm_pool = ctx.enter_context(tc.tile_pool(name="sm", bufs=1))
        xn_pool = ctx.enter_context(tc.tile_pool(name="xn", bufs=2))
        ps_mm = ctx.enter_context(tc.tile_pool(name="psmm", bufs=3, space="PSUM"))
        ps_s = ctx.enter_context(tc.tile_pool(name="pss", bufs=3, space="PSUM"))
        ps_o = ctx.enter_context(tc.tile_pool(name="pso", bufs=2, space="PSUM"))

        # ---------------- constants ----------------
        ident = const.tile([128, 128], F32, name="ident")
        make_identity(nc, ident[:])
        eps_t = const.tile([1, 1], F32, name="eps_t")
        nc.any.memset(eps_t[:], 1e-8)
        onesp = const.tile([128, 1], F32R, name="onesp")
        nc.sync.dma_start(onesp[:], onesp_d.ap().bitcast(F32R))
        onesb = const.tile([1, 128], F32R, name="onesb")
        nc.sync.dma_start(onesb[:], onesb_d.ap().bitcast(F32R))
        cosx = const.tile([128, S], F32, name="cosx")
        nc.sync.dma_start(cosx[:], cosx_d[:])
        sinx = const.tile([128, S], F32, name="sinx")
        nc.sync.dma_start(sinx[:], sinx_d[:])
        bqkv_sb = const.tile([128, L, 3], F32, name="bqkv_sb")
        nc.sync.dma_start(bqkv_sb[:], bqkv.ap().rearrange("l c p -> p l c"))
        bu_sb = const.tile([128, L, KT], F32, name="bu_sb")
        nc.sync.dma_start(bu_sb[:], bu.ap().rearrange("l m p -> p l m"))
        bt_sb = const.tile([128, L, KT], F32, name="bt_sb")
        nc.sync.dma_start(bt_sb[:], bt.ap().rearrange("l m p -> p l m"))
        lng_sb = const.tile([128, L, KT], F32, name="lng_sb")
        nc.sync.dma_start(lng_sb[:], lng.ap().rearrange("l m p -> p l m"))
        lnb_sb = const.tile([128, L, KT], F32, name="lnb_sb")
        nc.sync.dma_start(lnb_sb[:], lnb.ap().rearrange("l m p -> p l m"))
        lastg_sb = const.tile([128, KT], F32, name="lastg_sb")
        nc.sync.dma_start(lastg_sb[:], lastg.ap().rearrange("m p -> p m"))
        lastb_sb = const.tile([128, KT], F32, name="lastb_sb")
        nc.sync.dma_start(lastb_sb[:], lastb.ap().rearrange("m p -> p m"))
        bi32_sb = const.tile([128, KT], F32, name="bi32_sb")
        nc.sync.dma_start(bi32_sb[:], b_item32.ap().rearrange("m p -> p m"))
        bu32_sb = const.tile([128, KT], F32, name="bu32_sb")
        nc.sync.dma_start(bu32_sb[:], b_user32.ap().rearrange("m p -> p m"))
        bipl_sb = const.tile([128, KT], F32, name="bipl_sb")
        nc.sync.dma_start(bipl_sb[:], b_item_pl.ap().rearrange("m p -> p m"))

        def load_eT(src, tag):
            t = big.tile([128, KT, R], F32R, name=f"eT_{src}", tag=tag)
            nc.sync.dma_start(
                t[:], eT[src].ap().rearrange("(kt p) r -> p kt r", p=128).bitcast(F32R))
            return t

        def wstream(dram_2d, m, lview=None):
            """Load [128, KT, 128] k-major slice of a [H, H] weight (cols 128m..)."""
            t = ws_pool.tile([128, KT, 128], F32R, name="wsm", tag="wsm")
            ap = dram_2d if lview is None else dram_2d
            nc.sync.dma_start(
                t[:],
                ap.rearrange("(kt p) m -> p kt m", p=128)[:, :, 128 * m:128 * (m + 1)]
                .bitcast(F32R))
            return t

        # ---------------- phase 0 ----------------
        eT_item = load_eT("item", "tagA")
        eT_pos = load_eT("pos", "tagB")
        eT_neg = load_eT("neg", "tagC")
        eT_user = load_eT("user", "tagD")
        xacc = big.tile([128, KT, R], F32, name="xacc", tag="tagF")

        for m in range(KT):
            wm = wstream(w_item.ap(), m)
            for src, et in (("item", eT_item), ("pos", eT_pos), ("neg", eT_neg)):
                ps = ps_mm.tile([128, 512], F32, name="ps0", tag="mm")
                for k in range(KT):
                    nc.tensor.matmul(ps[:], wm[:, k], et[:, k],
                                     start=(k == 0), stop=(k == KT - 1))
                if src == "item":
                    nc.scalar.activation(xacc[:, m], ps[:], AF.Relu,
                                         bias=bi32_sb[:, m:m + 1], scale=32.0)
                else:
                    ot = ev_pool.tile([128, 512], F32, name="evt", tag="ev")
                    nc.scalar.activation(ot[:], ps[:], AF.Relu,
                                         bias=bipl_sb[:, m:m + 1])
                    dst = pos_T if src == "pos" else neg_T
                    nc.sync.dma_start(dst.ap()[128 * m:128 * (m + 1), :], ot[:])
        for m in range(KT):
            wm = wstream(w_user.ap(), m)
            ps = ps_mm.tile([128, 512], F32, name="ps0u", tag="mm")
            for k in range(KT):
                nc.tensor.matmul(ps[:], wm[:, k], eT_user[:, k],
                                 start=(k == 0), stop=(k == KT - 1))
            ut = ev_pool.tile([128, 512], F32, name="evtu", tag="ev")
            nc.scalar.activation(ut[:], ps[:], AF.Relu,
                                 bias=bu32_sb[:, m:m + 1], scale=32.0)
            nc.vector.tensor_add(xacc[:, m], xacc[:, m], ut[:])
            nc.sync.dma_start(ag_in[0].ap()[128 * m:128 * (m + 1), :], xacc[:, m])
        nc.gpsimd.collective_compute(
            "AllGather", OP.bypass, replica_groups=RG,
            ins=[ag_in[0].ap()], outs=[x_all[0].ap()])
        if DEBUG:
            nc.sync.dma_start(dbg["x0"].ap(), x_all[0].ap())

        # ---------------- layers ----------------
        for l in range(L):
            # --- U^T (needs only own block ag_in[l]; overlaps prior AG) ---
            xblk = big.tile([128, KT, R], F32R, name=f"xblk{l}", tag="xblk")
            nc.sync.dma_start(
                xblk[:],
                ag_in[l].ap().rearrange("(kt p) r -> p kt r", p=128).bitcast(F32R))
            UsT = big.tile([128, KT, R], F32, name=f"UsT{l}", tag="tagD")
            for m in range(KT):
                wm = wstream(wu.ap()[l], m)
                ps = ps_mm.tile([128, 512], F32, name="psu", tag="mm")
                for k in range(KT):
                    nc.tensor.matmul(ps[:], wm[:, k], xblk[:, k],
                                     start=(k == 0), stop=(k == KT - 1))
                nc.scalar.activation(UsT[:, m], ps[:], AF.Silu,
                                     bias=bu_sb[:, l, m:m + 1])

            # --- QKV over all token blocks ---
            wq_sb = wq_pool.tile([128, KT, 384], F32R, name=f"wq{l}", tag="wq")
            nc.sync.dma_start(
                wq_sb[:],
                wqkv.ap()[l].rearrange("(kt p) m -> p kt m", p=128).bitcast(F32R))
            QrT = big.tile([128, T], BF16, name=f"QrT{l}", tag="tagA")
            KrT = big.tile([128, T], BF16, name=f"KrT{l}", tag="tagB")
            vnorm = big.tile([128, 32, 128], BF16, name=f"vn{l}", tag="tagC")

            for r in range(NC):
                ps3 = [ps_mm.tile([128, 512], F32, name=f"psq{m}", tag="mm")
                       for m in range(3)]
                for k in range(KT):
                    t = xs_pool.tile([128, 512], F32R, name="xk", tag="xk")
                    nc.sync.dma_start(
                        t[:], x_all[l].ap()[r][128 * k:128 * (k + 1), :].bitcast(F32R))
                    for m in range(3):  # 0=V 1=Q 2=K
                        nc.tensor.matmul(ps3[m][:], wq_sb[:, k, 128 * m:128 * (m + 1)],
                                         t[:], start=(k == 0), stop=(k == KT - 1))
                ts = slice(512 * r, 512 * (r + 1))
                for m in range(3):
                    ps = ps3[m]
                    tmp = ev_pool.tile([128, 512], F32, name="qevt", tag="ev")
                    nc.scalar.activation(tmp[:], ps[:], AF.Identity,
                                         bias=bqkv_sb[:, l, m:m + 1])
                    if m == 0:
                        km = ev_pool.tile([128, 512], F32, name="km", tag="ev")
                        nc.sync.dma_start(km[:], kmaskx_d.ap()[:, ts])
                        vm = vt_pool.tile([128, 512], F32, name="vm", tag="vm")
                        nc.vector.tensor_mul(vm[:], tmp[:], km[:])
                        for q in range(4):
                            pst = ps_s.tile([128, 128], F32, name="pst", tag="s")
                            nc.tensor.transpose(pst[:], vm[:, 128 * q:128 * (q + 1)],
                                                ident[:])
                            nc.vector.tensor_copy(vnorm[:, 4 * r + q], pst[:])
                    else:
                        dstT = QrT if m == 1 else KrT
                        pl = 512 * (r % 4)
                        rq = rt_pool.tile([128, 512], F32, name="rq", tag="rq")
                        for bp in (0, 64):
                            nc.scalar.mul(rq[bp:bp + 32, :], tmp[bp + 32:bp + 64, :],
                                          -1.0)
                            nc.vector.tensor_copy(rq[bp + 32:bp + 64, :],
                                                  tmp[bp:bp + 32, :])
                        t1 = rt_pool.tile([128, 512], F32, name="rt1", tag="rt1")
                        nc.vector.tensor_mul(t1[:], tmp[:], cosx[:, pl:pl + 512])
                        nc.vector.tensor_mul(rq[:], rq[:], sinx[:, pl:pl + 512])
                        nc.vector.tensor_add(dstT[:, ts], t1[:], rq[:])
            if DEBUG and l == 0:
                nc.sync.dma_start(dbg["qr0"].ap(), QrT[:].bitcast(F32))
                nc.sync.dma_start(dbg["kr0"].ap(), KrT[:].bitcast(F32))

            # --- attention ---
            for b in range(B):
                for j in range(4):
                    och = oc_pool.tile([128, 512], F32, name="och", tag="och")
                    pso = ps_o.tile([128, 512], F32, name="pso", tag="o")
                    nt = 4 * (j + 1)
                    qs = slice(S * b + 512 * j, S * b + 512 * (j + 1))
                    for t in range(nt):
                        ks = slice(S * b + 128 * t, S * b + 128 * (t + 1))
                        wsbs = []
                        for hh in range(2):
                            hs = slice(64 * hh, 64 * hh + 64)
                            pss = ps_s.tile([128, 512], F32, name="pss", tag="s")
                            nc.tensor.matmul(pss[:], KrT[hs, ks], QrT[hs, qs],
                                             start=True, stop=True)
                            wsb = wt_pool.tile([128, 512], BF16, name="wsb",
                                               tag="w")
                            nc.scalar.activation(wsb[:], pss[:], AF.Sigmoid,
                                                 scale=SCALE)
                            d = t - 4 * j
                            if d >= 0:
                                # keep where qq - kk - 128d >= 0 else 0
                                nc.gpsimd.affine_select(
                                    wsb[:], wsb[:], pattern=[[1, 512]],
                                    compare_op=OP.is_ge, fill=0.0,
                                    base=-128 * d, channel_multiplier=-1)
                            wsbs.append(wsb)
                        for hh in range(2):
                            nc.tensor.matmul(
                                pso[64 * hh:64 * hh + 64, :],
                                vnorm[:, 16 * b + t, 64 * hh:64 * hh + 64],
                                wsbs[hh][:], start=(t == 0), stop=(t == nt - 1),
                                tile_position=(0, 64 * hh))
                    nc.vector.tensor_copy(och[:], pso[:])
                    nc.sync.dma_start(a2a_i[l].ap()[4 * b + j], och[:])
            nc.gpsimd.collective_compute(
                "AllToAll", OP.bypass, replica_groups=RG,
                ins=[a2a_i[l].ap()], outs=[a2a_o[l].ap()])
            if DEBUG and l == 0:
                nc.sync.dma_start(dbg["a2a0"].ap(), a2a_o[l].ap())

            # --- LN stats on o (channels across partitions x m) ---
            og = big.tile([128, KT, R], F32R, name=f"og{l}", tag="tagF")
            nc.sync.dma_start(
                og[:],
                a2a_o[l].ap().rearrange("c p r -> p c r").bitcast(F32R))
            ps_sum = ps_mm.tile([1, 512], F32, name="ps_sum", tag="mm")
            ps_sq = ps_mm.tile([1, 512], F32, name="ps_sq", tag="mm")
            for m in range(KT):
                sq = sm_pool.tile([128, 512], F32R, name="sq", tag="sq", bufs=2)
                nc.vector.tensor_mul(sq[:], og[:, m], og[:, m])
                nc.tensor.matmul(ps_sum[:], onesp[:], og[:, m],
                                 start=(m == 0), stop=(m == KT - 1))
                nc.tensor.matmul(ps_sq[:], onesp[:], sq[:],
                                 start=(m == 0), stop=(m == KT - 1))
            mean = sm_pool.tile([1, 512], F32, name="mean", tag="ln1")
            nc.scalar.mul(mean[:], ps_sum[:], 1.0 / H)
            ex2 = sm_pool.tile([1, 512], F32, name="ex2", tag="ln2")
            nc.scalar.mul(ex2[:], ps_sq[:], 1.0 / H)
            var = sm_pool.tile([1, 512], F32, name="var", tag="ln3")
            nc.vector.tensor_mul(var[:], mean[:], mean[:])
            nc.vector.tensor_sub(var[:], ex2[:], var[:])
            std = sm_pool.tile([1, 512], F32, name="std", tag="ln2")
            nc.scalar.activation(std[:], var[:], AF.Sqrt, bias=eps_t[:])
            rstd = sm_pool.tile([1, 512], F32, name="rstd", tag="ln3")
            nc.vector.reciprocal(rstd[:], std[:])
            stats2 = sm_pool.tile([1, 1024], F32R, name="stats2", tag="ln6")
            nc.vector.tensor_copy(stats2[:, 0:512], rstd[:])
            nc.vector.tensor_mul(stats2[:, 512:1024], mean[:], rstd[:])
            bps1 = ps_mm.tile([128, 512], F32, name="bps1", tag="mm")
            nc.tensor.matmul(bps1[:], onesb[:], stats2[:, 0:512], start=True,
                             stop=True)
            bps2 = ps_mm.tile([128, 512], F32, name="bps2", tag="mm")
            nc.tensor.matmul(bps2[:], onesb[:], stats2[:, 512:1024], start=True,
                             stop=True)
            bc_rstd = sm_pool.tile([128, 512], F32, name="bc_rstd", tag="bc1")
            nc.vector.tensor_copy(bc_rstd[:], bps1[:])
            bc_mr = sm_pool.tile([128, 512], F32, name="bc_mr", tag="bc2")
            nc.vector.tensor_copy(bc_mr[:], bps2[:])

            # --- normalize, gate (in place into og -> becomes gT) ---
            for m in range(KT):
                t1 = ev_pool.tile([128, 512], F32, name="lnt", tag="ev")
                nc.vector.tensor_mul(t1[:], og[:, m], bc_rstd[:])
                nc.vector.tensor_sub(t1[:], t1[:], bc_mr[:])
                nc.vector.tensor_scalar(t1[:], t1[:], lng_sb[:, l, m:m + 1],
                                        lnb_sb[:, l, m:m + 1], OP.mult, OP.add)
                nc.vector.tensor_mul(og[:, m], t1[:], UsT[:, m])
            if DEBUG and l == 0:
                nc.sync.dma_start(
                    dbg["g0"].ap().rearrange("m p r -> p m r"), og[:].bitcast(F32))

            # --- out^T + residual ---
            last = (l == L - 1)
            if last:
                xfin = big.tile([128, KT, R], F32R, name="xfin", tag="tagC")
            for m in range(KT):
                wm = wstream(wt.ap()[l], m)
                ps = ps_mm.tile([128, 512], F32, name="pst2", tag="mm")
                for k in range(KT):
                    nc.tensor.matmul(ps[:], wm[:, k], og[:, k],
                                     start=(k == 0), stop=(k == KT - 1))
                xn = xn_pool.tile([128, 512], F32, name="xn", tag="xn")
                nc.scalar.activation(xn[:], ps[:], AF.Identity,
                                     bias=bt_sb[:, l, m:m + 1])
                if last:
                    nc.vector.tensor_add(xfin[:, m], xn[:],
                                         xblk[:, m].bitcast(F32))
                else:
                    nc.vector.tensor_add(xn[:], xn[:], xblk[:, m].bitcast(F32))
                    nc.sync.dma_start(
                        ag_in[l + 1].ap()[128 * m:128 * (m + 1), :], xn[:])
            if not last:
                nc.gpsimd.collective_compute(
                    "AllGather", OP.bypass, replica_groups=RG,
                    ins=[ag_in[l + 1].ap()], outs=[x_all[l + 1].ap()])
                if DEBUG and l == 0:
                    nc.sync.dma_start(dbg["x1"].ap(), ag_in[1].ap())

        # ---------------- final layernorm ----------------
        ps_sum = ps_mm.tile([1, 512], F32, name="fsum", tag="mm")
        ps_sq = ps_mm.tile([1, 512], F32, name="fsq", tag="mm")
        for m in range(KT):
            sq = sm_pool.tile([128, 512], F32R, name="fsqt", tag="sq", bufs=2)
            nc.vector.tensor_mul(sq[:], xfin[:, m], xfin[:, m])
            nc.tensor.matmul(ps_sum[:], onesp[:], xfin[:, m],
                             start=(m == 0), stop=(m == KT - 1))
            nc.tensor.matmul(ps_sq[:], onesp[:], sq[:],
                             start=(m == 0), stop=(m == KT - 1))
        mean = sm_pool.tile([1, 512], F32, name="fmean", tag="ln1")
        nc.scalar.mul(mean[:], ps_sum[:], 1.0 / H)
        ex2 = sm_pool.tile([1, 512], F32, name="fex2", tag="ln2")
        nc.scalar.mul(ex2[:], ps_sq[:], 1.0 / H)
        var = sm_pool.tile([1, 512], F32, name="fvar", tag="ln3")
        nc.vector.tensor_mul(var[:], mean[:], mean[:])
        nc.vector.tensor_sub(var[:], ex2[:], var[:])
        std = sm_pool.tile([1, 512], F32, name="fstd", tag="ln2")
        nc.scalar.activation(std[:], var[:], AF.Sqrt, bias=eps_t[:])
        rstd = sm_pool.tile([1, 512], F32, name="frstd", tag="ln3")
        nc.vector.reciprocal(rstd[:], std[:])
        stats2 = sm_pool.tile([1, 1024], F32R, name="fstats2", tag="ln6")
        nc.vector.tensor_copy(stats2[:, 0:512], rstd[:])
        nc.vector.tensor_mul(stats2[:, 512:1024], mean[:], rstd[:])
        bps1 = ps_mm.tile([128, 512], F32, name="fbps1", tag="mm")
        nc.tensor.matmul(bps1[:], onesb[:], stats2[:, 0:512], start=True, stop=True)
        bps2 = ps_mm.tile([128, 512], F32, name="fbps2", tag="mm")
        nc.tensor.matmul(bps2[:], onesb[:], stats2[:, 512:1024], start=True,
                         stop=True)
        bc_rstd = sm_pool.tile([128, 512], F32, name="fbc_rstd", tag="bc1")
        nc.vector.tensor_copy(bc_rstd[:], bps1[:])
        bc_mr = sm_pool.tile([128, 512], F32, name="fbc_mr", tag="bc2")
        nc.vector.tensor_copy(bc_mr[:], bps2[:])
        for m in range(KT):
            t1 = ev_pool.tile([128, 512], F32, name="flnt", tag="ev")
            nc.vector.tensor_mul(t1[:], xfin[:, m], bc_rstd[:])
            nc.vector.tensor_sub(t1[:], t1[:], bc_mr[:])
            nc.vector.tensor_scalar(t1[:], t1[:], lastg_sb[:, m:m + 1],
                                    lastb_sb[:, m:m + 1], OP.mult, OP.add)
            nc.sync.dma_start(log_T.ap()[128 * m:128 * (m + 1), :], t1[:])

    nc.compile()
    return nc


def _host_prep(inputs):
    """Build per-core in_maps from full inputs."""
    inp = {k: np.asarray(v) for k, v in inputs.items()}
    item_emb = np.ascontiguousarray(inp["item_emb"], np.float32)
    user_emb = np.ascontiguousarray(inp["user_emb"], np.float32)
    mask = inp["mask"]
    ui = inp["user_item"]
    idx_item = np.clip(np.where(mask == 1, ui, 0), 0, item_emb.shape[0] - 1).reshape(T)
    idx_user = np.clip(np.where(mask == 2, ui, 0), 0, user_emb.shape[0] - 1).reshape(T)
    idx_pos = np.clip(inp["pos_seqs"], 0, item_emb.shape[0] - 1).reshape(T)
    idx_neg = np.clip(inp["neg_seqs"], 0, item_emb.shape[0] - 1).reshape(T)

    f32 = lambda x: np.ascontiguousarray(x, np.float32)
    proj_w = f32(inp["proj_w"])     # [L, H, 4H]
    proj_b = f32(inp["proj_b"])     # [L, 4H]
    wqkv = np.empty((L, H, 384), np.float32)
    bqkv = np.empty((L, 3, 128), np.float32)

    pos_of_t = np.arange(S)
    inv_freq = 1.0 / (10000.0 ** (np.arange(0, HD, 2, dtype=np.float64) / HD))
    freqs = pos_of_t[None, :] * inv_freq[:, None]          # [32, S]
    cos32 = np.cos(freqs).astype(np.float32)
    sin32 = np.sin(freqs).astype(np.float32)
    cosx = np.tile(cos32, (4, 1))                           # [128, S]
    sinx = np.tile(sin32, (4, 1))
    kmaskx = np.broadcast_to(
        (mask.reshape(T) != 0).astype(np.float32)[None, :], (128, T)).copy()
    shared = {
        "w_item": f32(inp["itemdnn_w"]),
        "w_user": f32(inp["userdnn_w"]),
        "b_item32": f32(32.0 * inp["itemdnn_b"]).reshape(KT, 128),
        "b_user32": f32(32.0 * inp["userdnn_b"]).reshape(KT, 128),
        "b_item_pl": f32(inp["itemdnn_b"]).reshape(KT, 128),
        "wu": np.ascontiguousarray(proj_w[:, :, :H]),
        "bu": proj_b[:, :H].reshape(L, KT, 128).copy(),
        "wt": f32(inp["tr_w"]),
        "bt": f32(inp["tr_b"]).reshape(L, KT, 128),
        "lng": f32(inp["ln_g"]).reshape(L, KT, 128),
        "lnb": f32(inp["ln_b"]).reshape(L, KT, 128),
        "lastg": f32(inp["last_g"]).reshape(KT, 128),
        "lastb": f32(inp["last_b"]).reshape(KT, 128),
        "cosx": cosx, "sinx": sinx, "kmaskx": kmaskx,
        "onesp": np.ones((128, 1), np.float32),
        "onesb": np.ones((1, 128), np.float32),
    }

    in_maps = []
    for c in range(NC):
        for part in range(3):   # V, Q, K blocks for this core's heads
            base = H * (part + 1) + 128 * c
            wqkv[:, :, 128 * part:128 * (part + 1)] = proj_w[:, :, base:base + 128]
            bqkv[:, part, :] = proj_b[:, base:base + 128]
        sl = slice(c * R, (c + 1) * R)
        m = dict(shared)
        m["wqkv"] = wqkv.copy()
        m["bqkv"] = bqkv.copy()
        m["item_eT"] = np.ascontiguousarray(item_emb[idx_item[sl]].T)
        m["user_eT"] = np.ascontiguousarray(user_emb[idx_user[sl]].T)
        m["pos_eT"] = np.ascontiguousarray(item_emb[idx_pos[sl]].T)
        m["neg_eT"] = np.ascontiguousarray(item_emb[idx_neg[sl]].T)
        in_maps.append(m)
    return in_maps


# ---------------- runner (jit built once, supports repeat timing) ----------------

def _get_runner():
    if "runner" in _CACHE:
        return _CACHE["runner"]
    import jax
    from jax.sharding import Mesh, PartitionSpec
    from concourse import bass2jax
    from concourse.bass2jax import (_bass_exec_p, install_neuronx_cc_hook,
                                    partition_id_tensor)
    import concourse.mybir as mybir_

    nc = _build_module()
    _CACHE["nc"] = nc
    install_neuronx_cc_hook()

    partition_name = (nc.partition_id_tensor.name
                      if nc.partition_id_tensor else None)
    in_names, out_names, out_avals = [], [], []
    for alloc in nc.m.functions[0].allocations:
        if not isinstance(alloc, mybir_.MemoryLocationSet):
            continue
        name = alloc.memorylocations[0].name
        if alloc.kind == "ExternalInput":
            if name != partition_name:
                in_names.append(name)
        elif alloc.kind == "ExternalOutput":
            out_names.append(name)
            out_avals.append(jax.core.ShapedArray(
                tuple(alloc.tensor_shape), mybir_.dt.np(alloc.dtype)))
    n_params = len(in_names)
    all_names = list(in_names) + out_names
    if partition_name is not None:
        all_names.append(partition_name)

    def _body(*args):
        operands = list(args)
        if partition_name is not None:
            operands.append(partition_id_tensor())
        outs = _bass_exec_p.bind(
            *operands,
            out_avals=tuple(out_avals),
            in_names=tuple(all_names),
            out_names=tuple(out_names),
            lowering_input_output_aliases=(),
            sim_require_finite=True,
            sim_require_nnan=True,
            nc=nc,
        )
        return tuple(outs)

    devices = jax.devices()[:NC]
    mesh = Mesh(np.asarray(devices), ("core",))
    n_outs = len(out_names)
    sharded = jax.jit(
        jax.shard_map(_body, mesh=mesh,
                      in_specs=(PartitionSpec("core"),) * (n_params + n_outs),
                      out_specs=(PartitionSpec("core"),) * n_outs,
                      check_vma=False),
        donate_argnums=tuple(range(n_params, n_params + n_outs)),
        keep_unused=True)

    runner = (sharded, in_names, out_names, out_avals, n_params, mesh)
    _CACHE["runner"] = runner
    return runner


def _run(in_maps, n_timing_iters=0):
    import jax
    from jax.sharding import NamedSharding, PartitionSpec
    sharded, in_names, out_names, out_avals, n_params, mesh = _get_runner()
    concat_in = [
        np.concatenate([np.ascontiguousarray(in_maps[c][nm]) for c in range(NC)], 0)
        for nm in in_names]
    zeros = [np.zeros((NC * a.shape[0], *a.shape[1:]), a.dtype) for a in out_avals]
    out = sharded(*concat_in, *zeros)
    jax.block_until_ready(out)
    best_ns = None
    if n_timing_iters:
        sh = NamedSharding(mesh, PartitionSpec("core"))
        dev_in = [jax.device_put(x, sh) for x in concat_in]
        jax.block_until_ready(dev_in)
        times = []
        for _ in range(n_timing_iters):
            z = [jax.device_put(np.zeros((NC * a.shape[0], *a.shape[1:]), a.dtype),
                                sh)
                 for a in out_avals]
            jax.block_until_ready(z)
            t0 = time.perf_counter()
            o2 = sharded(*dev_in, *z)
            jax.block_until_ready(o2)
            times.append(time.perf_counter() - t0)
        print("timing samples ms:", [f"{t*1e3:.2f}" for t in times])
        best_ns = int(min(times) * 1e9)
    results = [
        {nm: np.asarray(out[i]).reshape(NC, *out_avals[i].shape)[c]
         for i, nm in enumerate(out_names)}
        for c in range(NC)]
    return results, best_ns


def _assemble(results, inputs):
    log = np.concatenate([r["log_T"].T for r in results], 0).reshape(B, S, H)
    pos = np.concatenate([r["pos_T"].T for r in results], 0).reshape(B, S, H)
    neg = np.concatenate([r["neg_T"].T for r in results], 0).reshape(B, S, H)
    loss_mask = np.asarray(inputs["next_mask"]) == 1
    return log, pos, neg, loss_mask


def kernel(**inputs):
    in_maps = _host_prep(inputs)
    results, _ = _run(in_maps)
    return _assemble(results, inputs)


def kernel_timed(n_iters=5, **inputs):
    in_maps = _host_prep(inputs)
    results, best_ns = _run(in_maps, n_timing_iters=n_iters)
    return _assemble(results, inputs), best_ns, results
